# revision 25
# baseline (speedup 1.0000x reference)
import sys

import numpy as np

for _p in ("/opt/trn_rl_repo",):
    if _p not in sys.path:
        sys.path.insert(0, _p)

import concourse.bass as bass
import concourse.mybir as mybir
from concourse import bacc
import concourse.tile as tile
from concourse import masks
from concourse.bass_utils import run_bass_kernel_spmd

B, N, E, H, DH = 64, 197, 768, 12, 64
NCORES = 8
BPC = B // NCORES  # batches per core
EPS = 1e-6
F32 = mybir.dt.float32
F16 = mybir.dt.float16
BF16 = mybir.dt.bfloat16

# token partition tiles (all 197 tokens incl cls)
TOK = ((0, 128), (128, 69))
GROUPS = BPC // 2  # 2 batches per group
GW = 2 * N  # 394
AF = mybir.ActivationFunctionType


def build_nc():
    nc = bacc.Bacc()
    xc = nc.declare_dram_parameter("xc", [BPC, N, E], F32, isOutput=False)
    wq = nc.declare_dram_parameter("wq", [E, E], BF16, isOutput=False)
    wk = nc.declare_dram_parameter("wk", [E, E], BF16, isOutput=False)
    wv = nc.declare_dram_parameter("wv", [E, E], BF16, isOutput=False)
    wva = nc.declare_dram_parameter("wva", [E, 36], BF16, isOutput=False)
    # l6[h] = L6 block at rows 6h..6h+5, zeros elsewhere (K=72 lhsT variants,
    # sidesteps the PE base-partition-must-be-0/32/64 rule)
    l6 = nc.declare_dram_parameter("l6", [H, 72, N], F16, isOutput=False)
    p2 = nc.declare_dram_parameter("p2", [N, 4], F32, isOutput=False)
    bias3 = nc.declare_dram_parameter("bias3", [128, 36], F32, isOutput=False)
    outc = nc.declare_dram_parameter("outc", [BPC, N, E], F32, isOutput=True)

    with tile.TileContext(nc) as tc:
        from contextlib import ExitStack

        with ExitStack() as ctx:
            ep = ctx.enter_context

            wpool = ep(tc.tile_pool(name="w", bufs=1))
            wrawpool = ep(tc.tile_pool(name="wraw", bufs=2))
            cpool = ep(tc.tile_pool(name="const", bufs=1))
            trawpool = ep(tc.tile_pool(name="traw", bufs=2))
            xTpool = ep(tc.tile_pool(name="xT", bufs=2))
            qkpool = ep(tc.tile_pool(name="qk", bufs=2))
            vpool = ep(tc.tile_pool(name="v", bufs=2))
            spool = ep(tc.tile_pool(name="small", bufs=2))
            rpool = ep(tc.tile_pool(name="r", bufs=4))
            btpool = ep(tc.tile_pool(name="bt", bufs=3))
            epool = ep(tc.tile_pool(name="e", bufs=3))
            opool = ep(tc.tile_pool(name="out", bufs=2))

            # PSUM banks: big 2 + arg 2 + av 4 (one per bi x it tag) = 8
            ps_big = ep(tc.tile_pool(name="ps_big", bufs=2, space="PSUM"))
            ps_arg = ep(tc.tile_pool(name="ps_arg", bufs=2, space="PSUM"))
            ps_av = ep(tc.tile_pool(name="ps_av", bufs=1, space="PSUM"))

            # ---- constants ----
            identf = cpool.tile([128, 128], F32, tag="identf")
            masks.make_identity(nc, identf[:, :])
            nc.vector.tensor_scalar_add(identf[:, :], identf[:, :], 0.0)
            identb = cpool.tile([128, 128], BF16, tag="identb")
            masks.make_identity(nc, identb[:, :])
            nc.vector.tensor_scalar_add(identb[:, :], identb[:, :], 0.0)
            identh = cpool.tile([128, 128], F16, tag="identh")
            masks.make_identity(nc, identh[:, :])
            nc.vector.tensor_scalar_add(identh[:, :], identh[:, :], 0.0)

            wq_t, wk_t, wv_t = [], [], []
            for name, dram, lst in (("q", wq, wq_t), ("k", wk, wk_t), ("v", wv, wv_t)):
                for ke in range(6):
                    traw = wrawpool.tile([128, E], BF16, tag="wraw", name="wraw")
                    nc.gpsimd.dma_start(traw[:, :], dram[ke * 128 : (ke + 1) * 128, :])
                    # stage through DVE so matmuls wait on DVE, not DMA queues
                    t = wpool.tile([128, E], BF16, tag=f"w{name}{ke}", name=f"w{name}{ke}")
                    nc.vector.tensor_scalar_add(t[:, :], traw[:, :], 0.0)
                    lst.append(t)
            wva_t = []
            for ke in range(6):
                traw = cpool.tile([128, 36], BF16, tag=f"wvar{ke}", name=f"wvar{ke}")
                nc.gpsimd.dma_start(traw[:, :], wva[ke * 128 : (ke + 1) * 128, :])
                t = cpool.tile([128, 36], BF16, tag=f"wva{ke}", name=f"wva{ke}")
                nc.vector.tensor_scalar_add(t[:, :], traw[:, :], 0.0)
                wva_t.append(t)
            l6_t = []
            for h in range(H):
                l6r = cpool.tile([72, N], F16, tag=f"l6r{h}", name=f"l6r{h}")
                nc.gpsimd.dma_start(l6r[:, :], l6[h, :, :])
                t = cpool.tile([72, N], F16, tag=f"l6t{h}", name=f"l6t{h}")
                nc.vector.tensor_scalar_add(t[:, :], l6r[:, :], 0.0)
                l6_t.append(t)
            p2_t = []
            for tt, (toff, tcnt) in enumerate(TOK):
                t = cpool.tile([128, 4], F32, tag=f"p2{tt}")
                nc.gpsimd.dma_start(t[:tcnt, :], p2[toff : toff + tcnt, :])
                p2_t.append(t)
            bias_t = cpool.tile([128, 36], F32, tag="bias3")
            nc.gpsimd.dma_start(bias_t[:, :], bias3[:, :])

            # ---- main loop over 2-batch groups ----
            for g in range(GROUPS):
                # --- load x and transpose to xT[eb] [128, 394] bf16 ---
                traw = [[None, None], [None, None]]
                for bi in range(2):
                    for tb, (toff, tcnt) in enumerate(TOK):
                        t = trawpool.tile([128, E], F32, tag=f"tr{bi}{tb}", name=f"tr{bi}{tb}")
                        nc.gpsimd.dma_start(t[:tcnt, :], xc[2 * g + bi, toff : toff + tcnt, :])
                        traw[bi][tb] = t
                xT = []
                for eb in range(6):
                    tp = ps_big.tile([128, GW], F32, tag="big", name="tpx")
                    first, last = (0, 0), (1, 1)
                    for bi in range(2):
                        for tb, (toff, tcnt) in enumerate(TOK):
                            nc.tensor.matmul(
                                tp[:128, bi * N + toff : bi * N + toff + tcnt],
                                traw[bi][tb][:tcnt, eb * 128 : (eb + 1) * 128],
                                identf[:tcnt, :tcnt],
                                is_transpose=True,
                                start=((bi, tb) == first),
                                stop=((bi, tb) == last),
                            )
                    t = xTpool.tile([128, GW], BF16, tag=f"xT{eb}", name=f"xT{eb}")
                    nc.vector.tensor_scalar_add(t[:, :], tp[:, :], 0.0)
                    xT.append(t)

                # --- q/k projections -> qTb/kTb [128, 394] bf16 (k prescaled 1/8) ---
                qTb, kTb = [], []
                for wt, lst, nm in ((wq_t, qTb, "q"), (wk_t, kTb, "k")):
                    for mo in range(6):
                        ps = ps_big.tile([128, GW], F32, tag="big", name="psqk")
                        for ke in range(6):
                            nc.tensor.matmul(
                                ps[:, :],
                                wt[ke][:, mo * 128 : (mo + 1) * 128],
                                xT[ke][:, :],
                                start=(ke == 0),
                                stop=(ke == 5),
                            )
                        t = qkpool.tile([128, GW], BF16, tag=f"{nm}T{mo}", name=f"{nm}T{mo}")
                        nc.vector.tensor_scalar_add(t[:, :], ps[:, :], 0.0)
                        lst.append(t)

                # --- v natural layout with interleaved ones col: [tok, 12*65] bf16 ---
                v_sb = [[None, None], [None, None]]
                for bi in range(2):
                    for tb, (toff, tcnt) in enumerate(TOK):
                        t = vpool.tile([128, H * 65], BF16, tag=f"v{bi}{tb}", name=f"v{bi}{tb}")
                        tv = t[:tcnt, :].rearrange("p (h c) -> p h c", c=65)
                        for nb in range(2):
                            ps = ps_arg.tile([128, 384], F32, tag="arg", name="psv")
                            for ke in range(6):
                                nc.tensor.matmul(
                                    ps[:tcnt, :],
                                    xT[ke][:, bi * N + toff : bi * N + toff + tcnt],
                                    wv_t[ke][:, nb * 384 : (nb + 1) * 384],
                                    start=(ke == 0),
                                    stop=(ke == 5),
                                )
                            nc.vector.tensor_scalar_add(
                                tv[:, nb * 6 : (nb + 1) * 6, 0:64],
                                ps[:tcnt, :].rearrange("p (h c) -> p h c", c=64),
                                0.0,
                            )
                        nc.gpsimd.memset(tv[:, :, 64:65], 1.0)
                        v_sb[bi][tb] = t

                # --- gaussian params -> R_T[bi] [72, 197] f16 (rows 6h+k) ---
                # ACT work is phase-batched (all Exp, then all Ln) to avoid
                # activation-table reloads between Exp and Ln.
                BP = [(bi, pt) for bi in range(2) for pt in range(2)]
                spe_t, sp_t, lna_t = {}, {}, {}
                for bi, pt in BP:
                    poff, pcnt = TOK[pt]
                    p36 = ps_arg.tile([128, 36], F32, tag="arg", name="p36")
                    for ke in range(6):
                        nc.tensor.matmul(
                            p36[:pcnt, :],
                            qTb[ke][:, bi * N + poff : bi * N + poff + pcnt],
                            wva_t[ke][:, :],
                            start=(ke == 0),
                            stop=(ke == 5),
                        )
                    # softplus(x) = ln(1 + exp(x))
                    spa = spool.tile([128, 36], F32, tag=f"spa{bi}{pt}")
                    nc.vector.tensor_add(spa[:pcnt, :], p36[:pcnt, :], bias_t[:pcnt, :])
                    spe = spool.tile([128, 36], F32, tag=f"spe{bi}{pt}")
                    nc.scalar.activation(spe[:pcnt, :], spa[:pcnt, :], AF.Exp)
                    spe_t[bi, pt] = spe
                for bi, pt in BP:
                    poff, pcnt = TOK[pt]
                    sp = spool.tile([128, 36], F32, tag=f"sp{bi}{pt}")
                    nc.scalar.activation(sp[:pcnt, :], spe_t[bi, pt][:pcnt, :], AF.Ln, bias=1.0)
                    sp_t[bi, pt] = sp
                    lna = spool.tile([128, 12], F32, tag=f"lna{bi}{pt}")
                    sp3 = sp[:pcnt, :].rearrange("p (h c) -> p h c", c=3)
                    nc.scalar.activation(lna[:pcnt, :].unsqueeze(2), sp3[:, :, 2:3], AF.Ln)
                    lna_t[bi, pt] = lna
                R_T = []
                for bi in range(2):
                    rtps = ps_arg.tile([72, N], F16, tag="arg", name="rtps")
                    for pt, (poff, pcnt) in enumerate(TOK):
                        sp3 = sp_t[bi, pt][:pcnt, :].rearrange("p (h c) -> p h c", c=3)
                        lna = lna_t[bi, pt]
                        # rv[p, 2h+c] = 1/(softplus + 2eps)
                        rv = spool.tile([128, 24], F32, tag="rv")
                        rv3 = rv[:pcnt, :].rearrange("p (h c) -> p h c", c=2)
                        nc.vector.tensor_scalar_add(rv3, sp3[:, :, 0:2], 2.0 * EPS)
                        nc.vector.reciprocal(rv[:pcnt, :], rv[:pcnt, :])
                        rvx = rv3[:, :, 0:1]
                        rvy = rv3[:, :, 1:2]
                        # R rows per head: [lna-0.5(rvx*px^2+rvy*py^2), rvx*px, -0.5rvx,
                        #                   rvy*py, -0.5rvy, -40]
                        px = p2_t[pt][:pcnt, 0:1]
                        px2 = p2_t[pt][:pcnt, 1:2]
                        py = p2_t[pt][:pcnt, 2:3]
                        py2 = p2_t[pt][:pcnt, 3:4]
                        rpre = rpool.tile([128, 72], F16, tag="rpre")
                        r6 = rpre[:pcnt, :].rearrange("p (h k) -> p h k", k=6)
                        nc.gpsimd.tensor_scalar_mul(r6[:, :, 1:2], rvx, px)
                        nc.gpsimd.tensor_scalar_mul(r6[:, :, 3:4], rvy, py)
                        nc.gpsimd.tensor_scalar_mul(r6[:, :, 2:3], rvx, -0.5)
                        nc.gpsimd.tensor_scalar_mul(r6[:, :, 4:5], rvy, -0.5)
                        ta = spool.tile([128, 12], F32, tag="ta")
                        tb2 = spool.tile([128, 12], F32, tag="tb2")
                        nc.gpsimd.tensor_scalar_mul(ta[:pcnt, :].unsqueeze(2), rvx, px2)
                        nc.gpsimd.tensor_scalar_mul(tb2[:pcnt, :].unsqueeze(2), rvy, py2)
                        tc2 = spool.tile([128, 12], F32, tag="tc2")
                        nc.gpsimd.tensor_add(tc2[:pcnt, :], ta[:pcnt, :], tb2[:pcnt, :])
                        nc.gpsimd.tensor_scalar_mul(tc2[:pcnt, :], tc2[:pcnt, :], -0.5)
                        nc.gpsimd.tensor_add(
                            r6[:, :, 0:1], tc2[:pcnt, :].unsqueeze(2), lna[:pcnt, :].unsqueeze(2)
                        )
                        nc.gpsimd.memset(r6[:, :, 5:6], -40.0)
                        if pt == 0:
                            # cls query col: zero linear terms, force R0 (and keep
                            # R5) at -40 so bias underflows to 0 for i=0 and (0,0)
                            r60 = rpre[0:1, :].rearrange("p (h k) -> p h k", k=6)
                            nc.gpsimd.memset(r60[:, :, 0:5], 0.0)
                            nc.gpsimd.memset(r60[:, :, 0:1], -40.0)
                        nc.tensor.matmul(
                            rtps[:72, poff : poff + pcnt],
                            rpre[:pcnt, :72],
                            identh[:pcnt, :pcnt],
                            is_transpose=True,
                            start=(pt == 0),
                            stop=(pt == 1),
                        )
                    t = rpool.tile([72, N], F16, tag="rT", name="rT")
                    nc.vector.tensor_scalar_add(t[:, :], rtps[:, :], 0.0)
                    R_T.append(t)

                # --- attention ---
                # same-parity head pairs (h, h+2) share lhsT base partitions, so a
                # pair's scores fit one PSUM bank; batches interleave per pair to
                # keep the PE dense enough that HAM stays unthrottled.
                out_sb = [
                    [
                        opool.tile([128, E], F32, tag=f"o{bi}{it}", name=f"o{bi}{it}")
                        for it in range(2)
                    ]
                    for bi in range(2)
                ]
                for pg in range(2):  # parity groups: heads pg, pg+2, ..., pg+10
                    ro = 64 * pg
                    av = [
                        [
                            ps_av.tile([128, 6 * 65], F32, tag=f"av{bi}{it}", name=f"av{bi}{it}")
                            for it in range(2)
                        ]
                        for bi in range(2)
                    ]
                    for pk in range(3):  # pair (h0, h0+2) within parity group
                        h0 = 4 * pk + pg
                        for bi in range(2):
                            e_t = []
                            for jt, (joff, jcnt) in enumerate(TOK):
                                ps = ps_big.tile([128, GW], F32, tag="big", name="pssc")
                                pa = ps_arg.tile([128, GW], F32, tag="arg", name="psarg")
                                for hh in range(2):
                                    h = h0 + 2 * hh
                                    mo = h // 2
                                    nc.tensor.matmul(
                                        ps[:jcnt, hh * N : (hh + 1) * N],
                                        kTb[mo][ro : ro + 64, bi * N + joff : bi * N + joff + jcnt],
                                        qTb[mo][ro : ro + 64, bi * N : bi * N + N],
                                        start=(hh == 0),
                                        stop=False,
                                    )
                                    nc.tensor.matmul(
                                        pa[:jcnt, hh * N : (hh + 1) * N],
                                        l6_t[h][:, joff : joff + jcnt],
                                        R_T[bi][:, :],
                                        start=(hh == 0),
                                        stop=(hh == 1),
                                    )
                                bt = btpool.tile([128, GW], BF16, tag="bt", name="bt")
                                nc.scalar.activation(bt[:jcnt, :], pa[:jcnt, :], AF.Exp)
                                nc.tensor.matmul(
                                    ps[:jcnt, :],
                                    identb[:jcnt, :jcnt],
                                    bt[:jcnt, :],
                                    start=False,
                                    stop=True,
                                )
                                e = epool.tile([128, GW], BF16, tag=f"e{jt}", name=f"e{jt}")
                                nc.scalar.activation(e[:jcnt, :], ps[:jcnt, :], AF.Exp)
                                e_t.append(e)
                            for it, (ioff, icnt) in enumerate(TOK):
                                for hh in range(2):
                                    h = h0 + 2 * hh
                                    col = (2 * pk + hh) * 65
                                    for jt, (joff, jcnt) in enumerate(TOK):
                                        nc.tensor.matmul(
                                            av[bi][it][:icnt, col : col + 65],
                                            e_t[jt][:jcnt, hh * N + ioff : hh * N + ioff + icnt],
                                            v_sb[bi][jt][:jcnt, h * 65 : h * 65 + 65],
                                            start=(pk == 0 and hh == 0 and jt == 0),
                                            stop=(pk == 2 and hh == 1 and jt == 1),
                                        )
                    # normalize 6 heads at once per (batch, token tile)
                    for bi in range(2):
                        for it, (ioff, icnt) in enumerate(TOK):
                            av3 = av[bi][it][:icnt, :].rearrange("p (h c) -> p h c", c=65)
                            rr = spool.tile([128, 6], F32, tag="rr")
                            nc.vector.reciprocal(rr[:icnt, :].unsqueeze(2), av3[:, :, 64:65])
                            ov = out_sb[bi][it][:icnt, :].rearrange(
                                "p (k two d) -> p k two d", two=2, d=64
                            )[:, :, pg, :]
                            nc.vector.tensor_mul(
                                ov,
                                av3[:, :, 0:64],
                                rr[:icnt, :].unsqueeze(2).broadcast_to([icnt, 6, 64]),
                            )
                for bi in range(2):
                    for it, (toff, tcnt) in enumerate(TOK):
                        nc.gpsimd.dma_start(
                            outc[2 * g + bi, toff : toff + tcnt, :], out_sb[bi][it][:tcnt, :]
                        )
    nc.compile()
    return nc


_NC_CACHE = None


def _get_nc():
    global _NC_CACHE
    if _NC_CACHE is None:
        _NC_CACHE = build_nc()
    return _NC_CACHE


def _prep_inputs(x, Wq, Wk, Wv, W_var, b_var, W_alpha, b_alpha, diff):
    import ml_dtypes

    bf16 = ml_dtypes.bfloat16
    x = np.asarray(x, np.float32)
    wq = np.ascontiguousarray(np.asarray(Wq, np.float32).T).astype(bf16)
    wk = np.ascontiguousarray(np.asarray(Wk, np.float32).T * 0.125).astype(bf16)
    wv = np.ascontiguousarray(np.asarray(Wv, np.float32).T).astype(bf16)
    W_var = np.asarray(W_var, np.float32)
    W_alpha = np.asarray(W_alpha, np.float32)
    diff = np.asarray(diff)
    # block-diagonal [768, 36]: cols 3h+{0,1,2} = W_var[0], W_var[1], W_alpha
    wva = np.zeros((E, 36), np.float32)
    for h in range(H):
        sl = slice(h * DH, (h + 1) * DH)
        wva[sl, 3 * h + 0] = W_var[0]
        wva[sl, 3 * h + 1] = W_var[1]
        wva[sl, 3 * h + 2] = W_alpha[0]
    wva = wva.astype(bf16)
    # grid coordinates per token (derived from diff against patch 0 at (0,0))
    pxp = np.sqrt(diff[:, 0, 0].astype(np.float64)).astype(np.float32)  # (196,)
    pyp = np.sqrt(diff[:, 0, 1].astype(np.float64)).astype(np.float32)
    px = np.concatenate([[0.0], pxp]).astype(np.float32)  # (197,) token-indexed
    py = np.concatenate([[0.0], pyp]).astype(np.float32)
    # L6 [6, 197]: col j>=1 -> [1, px, px^2, py, py^2, 0]; col 0 (cls) -> e_5
    l6a = np.zeros((6, N), np.float32)
    l6a[0, 1:] = 1.0
    l6a[1, 1:] = px[1:]
    l6a[2, 1:] = px[1:] ** 2
    l6a[3, 1:] = py[1:]
    l6a[4, 1:] = py[1:] ** 2
    l6a[5, 0] = 1.0
    # 12 block lhsT variants: l6[h] has L6 at rows 6h..6h+5, zeros elsewhere
    l6 = np.zeros((H, 72, N), np.float32)
    for h in range(H):
        l6[h, 6 * h : 6 * h + 6] = l6a
    l6 = l6.astype(np.float16)
    p2 = np.stack([px, px**2, py, py**2], axis=1).astype(np.float32)  # (197, 4)
    bias3 = np.tile(
        np.concatenate([np.asarray(b_var, np.float32), np.asarray(b_alpha, np.float32)]),
        (128, H),
    ).astype(np.float32)
    shared = dict(wq=wq, wk=wk, wv=wv, wva=wva, l6=l6, p2=p2, bias3=bias3)
    in_maps = []
    for c in range(NCORES):
        m = dict(shared)
        m["xc"] = np.ascontiguousarray(x[c * BPC : (c + 1) * BPC])
        in_maps.append(m)
    return in_maps


def run(trace=False, **inputs):
    nc = _get_nc()
    in_maps = _prep_inputs(**inputs)
    res = run_bass_kernel_spmd(nc, in_maps, list(range(NCORES)), trace=trace)
    out = np.concatenate([res.results[c]["outc"] for c in range(NCORES)], axis=0)
    return out, res


def kernel(**inputs):
    out, _ = run(trace=False, **inputs)
    return out


# revision 27
# speedup vs baseline: 1.1064x; 1.1064x over previous
import sys

import numpy as np

for _p in ("/opt/trn_rl_repo",):
    if _p not in sys.path:
        sys.path.insert(0, _p)

import concourse.bass as bass
import concourse.mybir as mybir
from concourse import bacc
import concourse.tile as tile
from concourse import masks
from concourse.bass_utils import run_bass_kernel_spmd

B, N, E, H, DH = 64, 197, 768, 12, 64
NCORES = 8
BPC = B // NCORES  # batches per core
EPS = 1e-6
F32 = mybir.dt.float32
F16 = mybir.dt.float16
BF16 = mybir.dt.bfloat16

# token partition tiles (all 197 tokens incl cls)
TOK = ((0, 128), (128, 69))
GROUPS = BPC // 2  # 2 batches per group
GW = 2 * N  # 394
AF = mybir.ActivationFunctionType


def build_nc():
    nc = bacc.Bacc()
    xc = nc.declare_dram_parameter("xc", [BPC, N, E], F32, isOutput=False)
    wq = nc.declare_dram_parameter("wq", [E, E], BF16, isOutput=False)
    wk = nc.declare_dram_parameter("wk", [E, E], BF16, isOutput=False)
    wv = nc.declare_dram_parameter("wv", [E, E], BF16, isOutput=False)
    wva = nc.declare_dram_parameter("wva", [E, 36], BF16, isOutput=False)
    # l6[h] = L6 block at rows 6h..6h+5, zeros elsewhere (K=72 lhsT variants,
    # sidesteps the PE base-partition-must-be-0/32/64 rule)
    l6 = nc.declare_dram_parameter("l6", [H, 72, N], F16, isOutput=False)
    p2 = nc.declare_dram_parameter("p2", [N, 4], F32, isOutput=False)
    bias3 = nc.declare_dram_parameter("bias3", [128, 36], F32, isOutput=False)
    outc = nc.declare_dram_parameter("outc", [BPC, N, E], F32, isOutput=True)

    with tile.TileContext(nc) as tc:
        from contextlib import ExitStack

        with ExitStack() as ctx:
            ep = ctx.enter_context

            wpool = ep(tc.tile_pool(name="w", bufs=1))
            wrawpool = ep(tc.tile_pool(name="wraw", bufs=2))
            cpool = ep(tc.tile_pool(name="const", bufs=1))
            trawpool = ep(tc.tile_pool(name="traw", bufs=2))
            xTpool = ep(tc.tile_pool(name="xT", bufs=2))
            qkpool = ep(tc.tile_pool(name="qk", bufs=2))
            vpool = ep(tc.tile_pool(name="v", bufs=2))
            spool = ep(tc.tile_pool(name="small", bufs=2))
            rpool = ep(tc.tile_pool(name="r", bufs=4))
            btpool = ep(tc.tile_pool(name="bt", bufs=3))
            epool = ep(tc.tile_pool(name="e", bufs=3))
            opool = ep(tc.tile_pool(name="out", bufs=2))

            # PSUM banks: big 2 + arg 2 + av 2x2 = 8
            ps_big = ep(tc.tile_pool(name="ps_big", bufs=2, space="PSUM"))
            ps_arg = ep(tc.tile_pool(name="ps_arg", bufs=2, space="PSUM"))
            ps_av = ep(tc.tile_pool(name="ps_av", bufs=2, space="PSUM"))

            # ---- constants ----
            identf = cpool.tile([128, 128], F32, tag="identf")
            masks.make_identity(nc, identf[:, :])
            nc.vector.tensor_scalar_add(identf[:, :], identf[:, :], 0.0)
            identb = cpool.tile([128, 128], BF16, tag="identb")
            masks.make_identity(nc, identb[:, :])
            nc.vector.tensor_scalar_add(identb[:, :], identb[:, :], 0.0)
            identh = cpool.tile([128, 128], F16, tag="identh")
            masks.make_identity(nc, identh[:, :])
            nc.vector.tensor_scalar_add(identh[:, :], identh[:, :], 0.0)

            wq_t, wk_t, wv_t = [], [], []
            for name, dram, lst in (("q", wq, wq_t), ("k", wk, wk_t), ("v", wv, wv_t)):
                for ke in range(6):
                    traw = wrawpool.tile([128, E], BF16, tag="wraw", name="wraw")
                    nc.gpsimd.dma_start(traw[:, :], dram[ke * 128 : (ke + 1) * 128, :])
                    # stage through DVE so matmuls wait on DVE, not DMA queues
                    t = wpool.tile([128, E], BF16, tag=f"w{name}{ke}", name=f"w{name}{ke}")
                    nc.vector.tensor_scalar_add(t[:, :], traw[:, :], 0.0)
                    lst.append(t)
            wva_t = []
            for ke in range(6):
                traw = cpool.tile([128, 36], BF16, tag=f"wvar{ke}", name=f"wvar{ke}")
                nc.gpsimd.dma_start(traw[:, :], wva[ke * 128 : (ke + 1) * 128, :])
                t = cpool.tile([128, 36], BF16, tag=f"wva{ke}", name=f"wva{ke}")
                nc.vector.tensor_scalar_add(t[:, :], traw[:, :], 0.0)
                wva_t.append(t)
            l6_t = []
            for h in range(H):
                l6r = cpool.tile([72, N], F16, tag=f"l6r{h}", name=f"l6r{h}")
                nc.gpsimd.dma_start(l6r[:, :], l6[h, :, :])
                t = cpool.tile([72, N], F16, tag=f"l6t{h}", name=f"l6t{h}")
                nc.vector.tensor_scalar_add(t[:, :], l6r[:, :], 0.0)
                l6_t.append(t)
            p2_t = []
            for tt, (toff, tcnt) in enumerate(TOK):
                t = cpool.tile([128, 4], F32, tag=f"p2{tt}")
                nc.gpsimd.dma_start(t[:tcnt, :], p2[toff : toff + tcnt, :])
                p2_t.append(t)
            bias_t = cpool.tile([128, 36], F32, tag="bias3")
            nc.gpsimd.dma_start(bias_t[:, :], bias3[:, :])

            # ---- main loop over 2-batch groups ----
            for g in range(GROUPS):
                # --- load x and transpose to xT[eb] [128, 394] bf16 ---
                traw = [[None, None], [None, None]]
                for bi in range(2):
                    for tb, (toff, tcnt) in enumerate(TOK):
                        t = trawpool.tile([128, E], F32, tag=f"tr{bi}{tb}", name=f"tr{bi}{tb}")
                        nc.gpsimd.dma_start(t[:tcnt, :], xc[2 * g + bi, toff : toff + tcnt, :])
                        traw[bi][tb] = t
                xT = []
                for eb in range(6):
                    tp = ps_big.tile([128, GW], F32, tag="big", name="tpx")
                    first, last = (0, 0), (1, 1)
                    for bi in range(2):
                        for tb, (toff, tcnt) in enumerate(TOK):
                            nc.tensor.matmul(
                                tp[:128, bi * N + toff : bi * N + toff + tcnt],
                                traw[bi][tb][:tcnt, eb * 128 : (eb + 1) * 128],
                                identf[:tcnt, :tcnt],
                                is_transpose=True,
                                start=((bi, tb) == first),
                                stop=((bi, tb) == last),
                            )
                    t = xTpool.tile([128, GW], BF16, tag=f"xT{eb}", name=f"xT{eb}")
                    nc.vector.tensor_scalar_add(t[:, :], tp[:, :], 0.0)
                    xT.append(t)

                # --- q/k projections -> qTb/kTb [128, 394] bf16 (k prescaled 1/8) ---
                qTb, kTb = [], []
                for wt, lst, nm in ((wq_t, qTb, "q"), (wk_t, kTb, "k")):
                    for mo in range(6):
                        ps = ps_big.tile([128, GW], F32, tag="big", name="psqk")
                        for ke in range(6):
                            nc.tensor.matmul(
                                ps[:, :],
                                wt[ke][:, mo * 128 : (mo + 1) * 128],
                                xT[ke][:, :],
                                start=(ke == 0),
                                stop=(ke == 5),
                            )
                        t = qkpool.tile([128, GW], BF16, tag=f"{nm}T{mo}", name=f"{nm}T{mo}")
                        nc.vector.tensor_scalar_add(t[:, :], ps[:, :], 0.0)
                        lst.append(t)

                # --- v natural layout with interleaved ones col: [tok, 12*65] bf16 ---
                v_sb = [[None, None], [None, None]]
                for bi in range(2):
                    for tb, (toff, tcnt) in enumerate(TOK):
                        t = vpool.tile([128, H * 65], BF16, tag=f"v{bi}{tb}", name=f"v{bi}{tb}")
                        tv = t[:tcnt, :].rearrange("p (h c) -> p h c", c=65)
                        for nb in range(2):
                            ps = ps_arg.tile([128, 384], F32, tag="arg", name="psv")
                            for ke in range(6):
                                nc.tensor.matmul(
                                    ps[:tcnt, :],
                                    xT[ke][:, bi * N + toff : bi * N + toff + tcnt],
                                    wv_t[ke][:, nb * 384 : (nb + 1) * 384],
                                    start=(ke == 0),
                                    stop=(ke == 5),
                                )
                            nc.vector.tensor_scalar_add(
                                tv[:, nb * 6 : (nb + 1) * 6, 0:64],
                                ps[:tcnt, :].rearrange("p (h c) -> p h c", c=64),
                                0.0,
                            )
                        nc.gpsimd.memset(tv[:, :, 64:65], 1.0)
                        v_sb[bi][tb] = t

                # --- gaussian params -> R_T[bi] [72, 197] f16 (rows 6h+k) ---
                # ACT work is phase-batched (all Exp, then all Ln) to avoid
                # activation-table reloads between Exp and Ln.
                BP = [(bi, pt) for bi in range(2) for pt in range(2)]
                spe_t, sp_t, lna_t = {}, {}, {}
                for bi, pt in BP:
                    poff, pcnt = TOK[pt]
                    p36 = ps_arg.tile([128, 36], F32, tag="arg", name="p36")
                    for ke in range(6):
                        nc.tensor.matmul(
                            p36[:pcnt, :],
                            qTb[ke][:, bi * N + poff : bi * N + poff + pcnt],
                            wva_t[ke][:, :],
                            start=(ke == 0),
                            stop=(ke == 5),
                        )
                    # softplus(x) = ln(1 + exp(x))
                    spa = spool.tile([128, 36], F32, tag=f"spa{bi}{pt}")
                    nc.vector.tensor_add(spa[:pcnt, :], p36[:pcnt, :], bias_t[:pcnt, :])
                    spe = spool.tile([128, 36], F32, tag=f"spe{bi}{pt}")
                    nc.scalar.activation(spe[:pcnt, :], spa[:pcnt, :], AF.Exp)
                    spe_t[bi, pt] = spe
                for bi, pt in BP:
                    poff, pcnt = TOK[pt]
                    sp = spool.tile([128, 36], F32, tag=f"sp{bi}{pt}")
                    nc.scalar.activation(sp[:pcnt, :], spe_t[bi, pt][:pcnt, :], AF.Ln, bias=1.0)
                    sp_t[bi, pt] = sp
                    lna = spool.tile([128, 12], F32, tag=f"lna{bi}{pt}")
                    sp3 = sp[:pcnt, :].rearrange("p (h c) -> p h c", c=3)
                    nc.scalar.activation(lna[:pcnt, :].unsqueeze(2), sp3[:, :, 2:3], AF.Ln)
                    lna_t[bi, pt] = lna
                R_T = []
                for bi in range(2):
                    rtps = ps_arg.tile([72, N], F16, tag="arg", name="rtps")
                    for pt, (poff, pcnt) in enumerate(TOK):
                        sp3 = sp_t[bi, pt][:pcnt, :].rearrange("p (h c) -> p h c", c=3)
                        lna = lna_t[bi, pt]
                        # rv[p, 2h+c] = 1/(softplus + 2eps)
                        rv = spool.tile([128, 24], F32, tag="rv")
                        rv3 = rv[:pcnt, :].rearrange("p (h c) -> p h c", c=2)
                        nc.vector.tensor_scalar_add(rv3, sp3[:, :, 0:2], 2.0 * EPS)
                        nc.vector.reciprocal(rv[:pcnt, :], rv[:pcnt, :])
                        rvx = rv3[:, :, 0:1]
                        rvy = rv3[:, :, 1:2]
                        # R rows per head: [lna-0.5(rvx*px^2+rvy*py^2), rvx*px, -0.5rvx,
                        #                   rvy*py, -0.5rvy, -40]
                        px = p2_t[pt][:pcnt, 0:1]
                        px2 = p2_t[pt][:pcnt, 1:2]
                        py = p2_t[pt][:pcnt, 2:3]
                        py2 = p2_t[pt][:pcnt, 3:4]
                        rpre = rpool.tile([128, 72], F16, tag="rpre")
                        r6 = rpre[:pcnt, :].rearrange("p (h k) -> p h k", k=6)
                        nc.gpsimd.tensor_scalar_mul(r6[:, :, 1:2], rvx, px)
                        nc.gpsimd.tensor_scalar_mul(r6[:, :, 3:4], rvy, py)
                        nc.gpsimd.tensor_scalar_mul(r6[:, :, 2:3], rvx, -0.5)
                        nc.gpsimd.tensor_scalar_mul(r6[:, :, 4:5], rvy, -0.5)
                        ta = spool.tile([128, 12], F32, tag="ta")
                        tb2 = spool.tile([128, 12], F32, tag="tb2")
                        nc.gpsimd.tensor_scalar_mul(ta[:pcnt, :].unsqueeze(2), rvx, px2)
                        nc.gpsimd.tensor_scalar_mul(tb2[:pcnt, :].unsqueeze(2), rvy, py2)
                        tc2 = spool.tile([128, 12], F32, tag="tc2")
                        nc.gpsimd.tensor_add(tc2[:pcnt, :], ta[:pcnt, :], tb2[:pcnt, :])
                        nc.gpsimd.tensor_scalar_mul(tc2[:pcnt, :], tc2[:pcnt, :], -0.5)
                        nc.gpsimd.tensor_add(
                            r6[:, :, 0:1], tc2[:pcnt, :].unsqueeze(2), lna[:pcnt, :].unsqueeze(2)
                        )
                        nc.gpsimd.memset(r6[:, :, 5:6], -40.0)
                        if pt == 0:
                            # cls query col: zero linear terms, force R0 (and keep
                            # R5) at -40 so bias underflows to 0 for i=0 and (0,0)
                            r60 = rpre[0:1, :].rearrange("p (h k) -> p h k", k=6)
                            nc.gpsimd.memset(r60[:, :, 0:5], 0.0)
                            nc.gpsimd.memset(r60[:, :, 0:1], -40.0)
                        nc.tensor.matmul(
                            rtps[:72, poff : poff + pcnt],
                            rpre[:pcnt, :72],
                            identh[:pcnt, :pcnt],
                            is_transpose=True,
                            start=(pt == 0),
                            stop=(pt == 1),
                        )
                    t = rpool.tile([72, N], F16, tag="rT", name="rT")
                    nc.vector.tensor_scalar_add(t[:, :], rtps[:, :], 0.0)
                    R_T.append(t)

                # --- attention ---
                # Same-parity head pairs (h, h+2) share lhsT base partitions, so a
                # pair's scores + bias accumulate fit one PSUM bank. All bias
                # tiles (exp of the rank-6 arg matmul) are produced up front so
                # the score loop's PE work never waits on the scalar engine.
                bt_t = {}
                for bi in range(2):
                    for pg in range(2):
                        for pk in range(3):
                            h0 = 4 * pk + pg
                            for jt, (joff, jcnt) in enumerate(TOK):
                                pa = ps_arg.tile([128, GW], F32, tag="arg", name="psarg")
                                for hh in range(2):
                                    h = h0 + 2 * hh
                                    nc.tensor.matmul(
                                        pa[:jcnt, hh * N : (hh + 1) * N],
                                        l6_t[h][:, joff : joff + jcnt],
                                        R_T[bi][:, :],
                                        start=(hh == 0),
                                        stop=(hh == 1),
                                    )
                                bt = btpool.tile(
                                    [128, GW], BF16, tag=f"bt{bi}{pg}{pk}{jt}",
                                    name="bt",
                                )
                                nc.scalar.activation(bt[:jcnt, :], pa[:jcnt, :], AF.Exp)
                                bt_t[bi, pg, pk, jt] = bt
                out_sb = [
                    [
                        opool.tile([128, E], F32, tag=f"o{bi}{it}", name=f"o{bi}{it}")
                        for it in range(2)
                    ]
                    for bi in range(2)
                ]
                for bi in range(2):
                    for pg in range(2):  # parity groups: heads pg, pg+2, ..., pg+10
                        ro = 64 * pg
                        av = [
                            ps_av.tile([128, 6 * 65], F32, tag=f"av{it}", name=f"av{it}")
                            for it in range(2)
                        ]

                        def av_block(pk, e_t):
                            h0 = 4 * pk + pg
                            for it, (ioff, icnt) in enumerate(TOK):
                                for hh in range(2):
                                    h = h0 + 2 * hh
                                    col = (2 * pk + hh) * 65
                                    for jt, (joff, jcnt) in enumerate(TOK):
                                        nc.tensor.matmul(
                                            av[it][:icnt, col : col + 65],
                                            e_t[jt][:jcnt, hh * N + ioff : hh * N + ioff + icnt],
                                            v_sb[bi][jt][:jcnt, h * 65 : h * 65 + 65],
                                            start=(pk == 0 and hh == 0 and jt == 0),
                                            stop=(pk == 2 and hh == 1 and jt == 1),
                                        )

                        prev = None
                        for pk in range(4):  # 3 pairs + AV lagged one pair
                            if pk < 3:
                                h0 = 4 * pk + pg
                                e_t = []
                                for jt, (joff, jcnt) in enumerate(TOK):
                                    ps = ps_big.tile([128, GW], F32, tag="big", name="pssc")
                                    for hh in range(2):
                                        h = h0 + 2 * hh
                                        mo = h // 2
                                        nc.tensor.matmul(
                                            ps[:jcnt, hh * N : (hh + 1) * N],
                                            kTb[mo][
                                                ro : ro + 64,
                                                bi * N + joff : bi * N + joff + jcnt,
                                            ],
                                            qTb[mo][ro : ro + 64, bi * N : bi * N + N],
                                            start=(hh == 0),
                                            stop=False,
                                        )
                                    nc.tensor.matmul(
                                        ps[:jcnt, :],
                                        identb[:jcnt, :jcnt],
                                        bt_t[bi, pg, pk, jt][:jcnt, :],
                                        start=False,
                                        stop=True,
                                    )
                                    e = epool.tile(
                                        [128, GW], BF16, tag=f"e{jt}", name=f"e{jt}"
                                    )
                                    nc.scalar.activation(e[:jcnt, :], ps[:jcnt, :], AF.Exp)
                                    e_t.append(e)
                            if pk >= 1:
                                av_block(*prev)
                            prev = (pk, e_t) if pk < 3 else None
                        # normalize 6 heads at once per token tile
                        for it, (ioff, icnt) in enumerate(TOK):
                            av3 = av[it][:icnt, :].rearrange("p (h c) -> p h c", c=65)
                            rr = spool.tile([128, 6], F32, tag="rr")
                            nc.vector.reciprocal(rr[:icnt, :].unsqueeze(2), av3[:, :, 64:65])
                            ov = out_sb[bi][it][:icnt, :].rearrange(
                                "p (k two d) -> p k two d", two=2, d=64
                            )[:, :, pg, :]
                            nc.vector.tensor_mul(
                                ov,
                                av3[:, :, 0:64],
                                rr[:icnt, :].unsqueeze(2).broadcast_to([icnt, 6, 64]),
                            )
                for bi in range(2):
                    for it, (toff, tcnt) in enumerate(TOK):
                        nc.gpsimd.dma_start(
                            outc[2 * g + bi, toff : toff + tcnt, :], out_sb[bi][it][:tcnt, :]
                        )
    nc.compile()
    return nc


_NC_CACHE = None


def _get_nc():
    global _NC_CACHE
    if _NC_CACHE is None:
        _NC_CACHE = build_nc()
    return _NC_CACHE


def _prep_inputs(x, Wq, Wk, Wv, W_var, b_var, W_alpha, b_alpha, diff):
    import ml_dtypes

    bf16 = ml_dtypes.bfloat16
    x = np.asarray(x, np.float32)
    wq = np.ascontiguousarray(np.asarray(Wq, np.float32).T).astype(bf16)
    wk = np.ascontiguousarray(np.asarray(Wk, np.float32).T * 0.125).astype(bf16)
    wv = np.ascontiguousarray(np.asarray(Wv, np.float32).T).astype(bf16)
    W_var = np.asarray(W_var, np.float32)
    W_alpha = np.asarray(W_alpha, np.float32)
    diff = np.asarray(diff)
    # block-diagonal [768, 36]: cols 3h+{0,1,2} = W_var[0], W_var[1], W_alpha
    wva = np.zeros((E, 36), np.float32)
    for h in range(H):
        sl = slice(h * DH, (h + 1) * DH)
        wva[sl, 3 * h + 0] = W_var[0]
        wva[sl, 3 * h + 1] = W_var[1]
        wva[sl, 3 * h + 2] = W_alpha[0]
    wva = wva.astype(bf16)
    # grid coordinates per token (derived from diff against patch 0 at (0,0))
    pxp = np.sqrt(diff[:, 0, 0].astype(np.float64)).astype(np.float32)  # (196,)
    pyp = np.sqrt(diff[:, 0, 1].astype(np.float64)).astype(np.float32)
    px = np.concatenate([[0.0], pxp]).astype(np.float32)  # (197,) token-indexed
    py = np.concatenate([[0.0], pyp]).astype(np.float32)
    # L6 [6, 197]: col j>=1 -> [1, px, px^2, py, py^2, 0]; col 0 (cls) -> e_5
    l6a = np.zeros((6, N), np.float32)
    l6a[0, 1:] = 1.0
    l6a[1, 1:] = px[1:]
    l6a[2, 1:] = px[1:] ** 2
    l6a[3, 1:] = py[1:]
    l6a[4, 1:] = py[1:] ** 2
    l6a[5, 0] = 1.0
    # 12 block lhsT variants: l6[h] has L6 at rows 6h..6h+5, zeros elsewhere
    l6 = np.zeros((H, 72, N), np.float32)
    for h in range(H):
        l6[h, 6 * h : 6 * h + 6] = l6a
    l6 = l6.astype(np.float16)
    p2 = np.stack([px, px**2, py, py**2], axis=1).astype(np.float32)  # (197, 4)
    bias3 = np.tile(
        np.concatenate([np.asarray(b_var, np.float32), np.asarray(b_alpha, np.float32)]),
        (128, H),
    ).astype(np.float32)
    shared = dict(wq=wq, wk=wk, wv=wv, wva=wva, l6=l6, p2=p2, bias3=bias3)
    in_maps = []
    for c in range(NCORES):
        m = dict(shared)
        m["xc"] = np.ascontiguousarray(x[c * BPC : (c + 1) * BPC])
        in_maps.append(m)
    return in_maps


def run(trace=False, **inputs):
    nc = _get_nc()
    in_maps = _prep_inputs(**inputs)
    res = run_bass_kernel_spmd(nc, in_maps, list(range(NCORES)), trace=trace)
    out = np.concatenate([res.results[c]["outc"] for c in range(NCORES)], axis=0)
    return out, res


def kernel(**inputs):
    out, _ = run(trace=False, **inputs)
    return out


# revision 29
# speedup vs baseline: 1.1642x; 1.0522x over previous
import sys

import numpy as np

for _p in ("/opt/trn_rl_repo",):
    if _p not in sys.path:
        sys.path.insert(0, _p)

import concourse.bass as bass
import concourse.mybir as mybir
from concourse import bacc
import concourse.tile as tile
from concourse import masks
from concourse.bass_utils import run_bass_kernel_spmd

B, N, E, H, DH = 64, 197, 768, 12, 64
NCORES = 8
BPC = B // NCORES  # batches per core
EPS = 1e-6
F32 = mybir.dt.float32
F16 = mybir.dt.float16
BF16 = mybir.dt.bfloat16

# token partition tiles (all 197 tokens incl cls)
TOK = ((0, 128), (128, 69))
GROUPS = BPC // 2  # 2 batches per group
GW = 2 * N  # 394
AF = mybir.ActivationFunctionType


def build_nc():
    nc = bacc.Bacc()
    xc = nc.declare_dram_parameter("xc", [BPC, N, E], F32, isOutput=False)
    wq = nc.declare_dram_parameter("wq", [E, E], BF16, isOutput=False)
    wk = nc.declare_dram_parameter("wk", [E, E], BF16, isOutput=False)
    wv = nc.declare_dram_parameter("wv", [E, E], BF16, isOutput=False)
    wva = nc.declare_dram_parameter("wva", [E, 36], BF16, isOutput=False)
    # l6[h] = L6 block at rows 6h..6h+5, zeros elsewhere (K=72 lhsT variants,
    # sidesteps the PE base-partition-must-be-0/32/64 rule)
    l6 = nc.declare_dram_parameter("l6", [H, 72, N], F16, isOutput=False)
    p2 = nc.declare_dram_parameter("p2", [N, 4], F32, isOutput=False)
    bias3 = nc.declare_dram_parameter("bias3", [128, 36], F32, isOutput=False)
    outc = nc.declare_dram_parameter("outc", [BPC, N, E], F32, isOutput=True)

    with tile.TileContext(nc) as tc:
        from contextlib import ExitStack

        with ExitStack() as ctx:
            ep = ctx.enter_context

            wpool = ep(tc.tile_pool(name="w", bufs=1))
            wrawpool = ep(tc.tile_pool(name="wraw", bufs=2))
            cpool = ep(tc.tile_pool(name="const", bufs=1))
            trawpool = ep(tc.tile_pool(name="traw", bufs=2))
            xTpool = ep(tc.tile_pool(name="xT", bufs=2))
            qkpool = ep(tc.tile_pool(name="qk", bufs=2))
            vpool = ep(tc.tile_pool(name="v", bufs=2))
            spool = ep(tc.tile_pool(name="small", bufs=2))
            rpool = ep(tc.tile_pool(name="r", bufs=4))
            btpool = ep(tc.tile_pool(name="bt", bufs=3))
            epool = ep(tc.tile_pool(name="e", bufs=3))
            opool = ep(tc.tile_pool(name="out", bufs=2))

            # PSUM banks: big 2 + arg 2 + av 2x2 = 8
            ps_big = ep(tc.tile_pool(name="ps_big", bufs=2, space="PSUM"))
            ps_arg = ep(tc.tile_pool(name="ps_arg", bufs=2, space="PSUM"))
            ps_av = ep(tc.tile_pool(name="ps_av", bufs=2, space="PSUM"))

            # ---- constants ----
            identf = cpool.tile([128, 128], F32, tag="identf")
            masks.make_identity(nc, identf[:, :])
            nc.vector.tensor_scalar_add(identf[:, :], identf[:, :], 0.0)
            identb = cpool.tile([128, 128], BF16, tag="identb")
            masks.make_identity(nc, identb[:, :])
            nc.vector.tensor_scalar_add(identb[:, :], identb[:, :], 0.0)
            identh = cpool.tile([128, 128], F16, tag="identh")
            masks.make_identity(nc, identh[:, :])
            nc.vector.tensor_scalar_add(identh[:, :], identh[:, :], 0.0)

            wq_t, wk_t, wv_t = [], [], []
            for name, dram, lst in (("q", wq, wq_t), ("k", wk, wk_t), ("v", wv, wv_t)):
                for ke in range(6):
                    traw = wrawpool.tile([128, E], BF16, tag="wraw", name="wraw")
                    nc.gpsimd.dma_start(traw[:, :], dram[ke * 128 : (ke + 1) * 128, :])
                    # stage through DVE so matmuls wait on DVE, not DMA queues
                    t = wpool.tile([128, E], BF16, tag=f"w{name}{ke}", name=f"w{name}{ke}")
                    nc.vector.tensor_scalar_add(t[:, :], traw[:, :], 0.0)
                    lst.append(t)
            wva_t = []
            for ke in range(6):
                traw = cpool.tile([128, 36], BF16, tag=f"wvar{ke}", name=f"wvar{ke}")
                nc.gpsimd.dma_start(traw[:, :], wva[ke * 128 : (ke + 1) * 128, :])
                t = cpool.tile([128, 36], BF16, tag=f"wva{ke}", name=f"wva{ke}")
                nc.vector.tensor_scalar_add(t[:, :], traw[:, :], 0.0)
                wva_t.append(t)
            l6_t = []
            for h in range(H):
                l6r = cpool.tile([72, N], F16, tag=f"l6r{h}", name=f"l6r{h}")
                nc.gpsimd.dma_start(l6r[:, :], l6[h, :, :])
                t = cpool.tile([72, N], F16, tag=f"l6t{h}", name=f"l6t{h}")
                nc.vector.tensor_scalar_add(t[:, :], l6r[:, :], 0.0)
                l6_t.append(t)
            p2_t = []
            for tt, (toff, tcnt) in enumerate(TOK):
                t = cpool.tile([128, 4], F32, tag=f"p2{tt}")
                nc.gpsimd.dma_start(t[:tcnt, :], p2[toff : toff + tcnt, :])
                p2_t.append(t)
            bias_t = cpool.tile([128, 36], F32, tag="bias3")
            nc.gpsimd.dma_start(bias_t[:, :], bias3[:, :])

            # ---- main loop over 2-batch groups ----
            for g in range(GROUPS):
                # --- load x and transpose to xT[eb] [128, 394] bf16 ---
                traw = [[None, None], [None, None]]
                for bi in range(2):
                    for tb, (toff, tcnt) in enumerate(TOK):
                        t = trawpool.tile([128, E], F32, tag=f"tr{bi}{tb}", name=f"tr{bi}{tb}")
                        nc.gpsimd.dma_start(t[:tcnt, :], xc[2 * g + bi, toff : toff + tcnt, :])
                        traw[bi][tb] = t
                xT = []
                for eb in range(6):
                    tp = ps_big.tile([128, GW], F32, tag="big", name="tpx")
                    first, last = (0, 0), (1, 1)
                    for bi in range(2):
                        for tb, (toff, tcnt) in enumerate(TOK):
                            nc.tensor.matmul(
                                tp[:128, bi * N + toff : bi * N + toff + tcnt],
                                traw[bi][tb][:tcnt, eb * 128 : (eb + 1) * 128],
                                identf[:tcnt, :tcnt],
                                is_transpose=True,
                                start=((bi, tb) == first),
                                stop=((bi, tb) == last),
                            )
                    t = xTpool.tile([128, GW], BF16, tag=f"xT{eb}", name=f"xT{eb}")
                    nc.vector.tensor_scalar_add(t[:, :], tp[:, :], 0.0)
                    xT.append(t)

                # --- q/k projections -> qTb/kTb [128, 394] bf16 (k prescaled 1/8) ---
                qTb, kTb = [], []
                for wt, lst, nm in ((wq_t, qTb, "q"), (wk_t, kTb, "k")):
                    for mo in range(6):
                        ps = ps_big.tile([128, GW], F32, tag="big", name="psqk")
                        for ke in range(6):
                            nc.tensor.matmul(
                                ps[:, :],
                                wt[ke][:, mo * 128 : (mo + 1) * 128],
                                xT[ke][:, :],
                                start=(ke == 0),
                                stop=(ke == 5),
                            )
                        t = qkpool.tile([128, GW], BF16, tag=f"{nm}T{mo}", name=f"{nm}T{mo}")
                        nc.vector.tensor_scalar_add(t[:, :], ps[:, :], 0.0)
                        lst.append(t)

                # --- gaussian params -> R_T[bi] [72, 197] f16 (rows 6h+k) ---
                # ACT work is phase-batched (all Exp, then all Ln) to avoid
                # activation-table reloads between Exp and Ln.
                BP = [(bi, pt) for bi in range(2) for pt in range(2)]
                spe_t, sp_t, lna_t = {}, {}, {}
                for bi, pt in BP:
                    poff, pcnt = TOK[pt]
                    p36 = ps_arg.tile([128, 36], F32, tag="arg", name="p36")
                    for ke in range(6):
                        nc.tensor.matmul(
                            p36[:pcnt, :],
                            qTb[ke][:, bi * N + poff : bi * N + poff + pcnt],
                            wva_t[ke][:, :],
                            start=(ke == 0),
                            stop=(ke == 5),
                        )
                    # softplus(x) = ln(1 + exp(x))
                    spa = spool.tile([128, 36], F32, tag=f"spa{bi}{pt}")
                    nc.vector.tensor_add(spa[:pcnt, :], p36[:pcnt, :], bias_t[:pcnt, :])
                    spe = spool.tile([128, 36], F32, tag=f"spe{bi}{pt}")
                    nc.scalar.activation(spe[:pcnt, :], spa[:pcnt, :], AF.Exp)
                    spe_t[bi, pt] = spe
                for bi, pt in BP:
                    poff, pcnt = TOK[pt]
                    sp = spool.tile([128, 36], F32, tag=f"sp{bi}{pt}")
                    nc.scalar.activation(sp[:pcnt, :], spe_t[bi, pt][:pcnt, :], AF.Ln, bias=1.0)
                    sp_t[bi, pt] = sp
                    lna = spool.tile([128, 12], F32, tag=f"lna{bi}{pt}")
                    sp3 = sp[:pcnt, :].rearrange("p (h c) -> p h c", c=3)
                    nc.scalar.activation(lna[:pcnt, :].unsqueeze(2), sp3[:, :, 2:3], AF.Ln)
                    lna_t[bi, pt] = lna
                R_T = []
                for bi in range(2):
                    rtps = ps_arg.tile([72, N], F16, tag="arg", name="rtps")
                    for pt, (poff, pcnt) in enumerate(TOK):
                        sp3 = sp_t[bi, pt][:pcnt, :].rearrange("p (h c) -> p h c", c=3)
                        lna = lna_t[bi, pt]
                        # rv[p, 2h+c] = 1/(softplus + 2eps)
                        rv = spool.tile([128, 24], F32, tag="rv")
                        rv3 = rv[:pcnt, :].rearrange("p (h c) -> p h c", c=2)
                        nc.vector.tensor_scalar_add(rv3, sp3[:, :, 0:2], 2.0 * EPS)
                        nc.vector.reciprocal(rv[:pcnt, :], rv[:pcnt, :])
                        rvx = rv3[:, :, 0:1]
                        rvy = rv3[:, :, 1:2]
                        # R rows per head: [lna-0.5(rvx*px^2+rvy*py^2), rvx*px, -0.5rvx,
                        #                   rvy*py, -0.5rvy, -40]
                        px = p2_t[pt][:pcnt, 0:1]
                        px2 = p2_t[pt][:pcnt, 1:2]
                        py = p2_t[pt][:pcnt, 2:3]
                        py2 = p2_t[pt][:pcnt, 3:4]
                        rpre = rpool.tile([128, 72], F16, tag="rpre")
                        r6 = rpre[:pcnt, :].rearrange("p (h k) -> p h k", k=6)
                        nc.vector.tensor_scalar_mul(r6[:, :, 1:2], rvx, px)
                        nc.vector.tensor_scalar_mul(r6[:, :, 3:4], rvy, py)
                        nc.vector.tensor_scalar_mul(r6[:, :, 2:3], rvx, -0.5)
                        nc.vector.tensor_scalar_mul(r6[:, :, 4:5], rvy, -0.5)
                        ta = spool.tile([128, 12], F32, tag="ta")
                        tb2 = spool.tile([128, 12], F32, tag="tb2")
                        nc.vector.tensor_scalar_mul(ta[:pcnt, :].unsqueeze(2), rvx, px2)
                        nc.vector.tensor_scalar_mul(tb2[:pcnt, :].unsqueeze(2), rvy, py2)
                        tc2 = spool.tile([128, 12], F32, tag="tc2")
                        nc.vector.tensor_add(tc2[:pcnt, :], ta[:pcnt, :], tb2[:pcnt, :])
                        nc.vector.tensor_scalar_mul(tc2[:pcnt, :], tc2[:pcnt, :], -0.5)
                        nc.vector.tensor_add(
                            r6[:, :, 0:1], tc2[:pcnt, :].unsqueeze(2), lna[:pcnt, :].unsqueeze(2)
                        )
                        nc.vector.memset(r6[:, :, 5:6], -40.0)
                        if pt == 0:
                            # cls query col: zero linear terms, force R0 (and keep
                            # R5) at -40 so bias underflows to 0 for i=0 and (0,0)
                            r60 = rpre[0:1, :].rearrange("p (h k) -> p h k", k=6)
                            nc.vector.memset(r60[:, :, 0:5], 0.0)
                            nc.vector.memset(r60[:, :, 0:1], -40.0)
                        nc.tensor.matmul(
                            rtps[:72, poff : poff + pcnt],
                            rpre[:pcnt, :72],
                            identh[:pcnt, :pcnt],
                            is_transpose=True,
                            start=(pt == 0),
                            stop=(pt == 1),
                        )
                    t = rpool.tile([72, N], F16, tag="rT", name="rT")
                    nc.vector.tensor_scalar_add(t[:, :], rtps[:, :], 0.0)
                    R_T.append(t)

                # --- v natural layout with interleaved ones col: [tok, 12*65] bf16 ---
                v_sb = [[None, None], [None, None]]
                for bi in range(2):
                    for tb, (toff, tcnt) in enumerate(TOK):
                        t = vpool.tile([128, H * 65], BF16, tag=f"v{bi}{tb}", name=f"v{bi}{tb}")
                        tv = t[:tcnt, :].rearrange("p (h c) -> p h c", c=65)
                        for nb in range(2):
                            ps = ps_arg.tile([128, 384], F32, tag="arg", name="psv")
                            for ke in range(6):
                                nc.tensor.matmul(
                                    ps[:tcnt, :],
                                    xT[ke][:, bi * N + toff : bi * N + toff + tcnt],
                                    wv_t[ke][:, nb * 384 : (nb + 1) * 384],
                                    start=(ke == 0),
                                    stop=(ke == 5),
                                )
                            nc.vector.tensor_scalar_add(
                                tv[:, nb * 6 : (nb + 1) * 6, 0:64],
                                ps[:tcnt, :].rearrange("p (h c) -> p h c", c=64),
                                0.0,
                            )
                        nc.vector.memset(tv[:, :, 64:65], 1.0)
                        v_sb[bi][tb] = t

                # --- attention ---
                # Same-parity head pairs (h, h+2) share lhsT base partitions, so a
                # pair's scores + bias accumulate fit one PSUM bank. All bias
                # tiles (exp of the rank-6 arg matmul) are produced up front so
                # the score loop's PE work never waits on the scalar engine.
                bt_t = {}
                for bi in range(2):
                    for pg in range(2):
                        for pk in range(3):
                            h0 = 4 * pk + pg
                            for jt, (joff, jcnt) in enumerate(TOK):
                                pa = ps_arg.tile([128, GW], F32, tag="arg", name="psarg")
                                for hh in range(2):
                                    h = h0 + 2 * hh
                                    nc.tensor.matmul(
                                        pa[:jcnt, hh * N : (hh + 1) * N],
                                        l6_t[h][:, joff : joff + jcnt],
                                        R_T[bi][:, :],
                                        start=(hh == 0),
                                        stop=(hh == 1),
                                    )
                                bt = btpool.tile(
                                    [128, GW], BF16, tag=f"bt{bi}{pg}{pk}{jt}",
                                    name="bt",
                                )
                                nc.scalar.activation(bt[:jcnt, :], pa[:jcnt, :], AF.Exp)
                                bt_t[bi, pg, pk, jt] = bt
                out_sb = [
                    [
                        opool.tile([128, E], F32, tag=f"o{bi}{it}", name=f"o{bi}{it}")
                        for it in range(2)
                    ]
                    for bi in range(2)
                ]
                for bi in range(2):
                    for pg in range(2):  # parity groups: heads pg, pg+2, ..., pg+10
                        ro = 64 * pg
                        av = [
                            ps_av.tile([128, 6 * 65], F32, tag=f"av{it}", name=f"av{it}")
                            for it in range(2)
                        ]

                        def av_block(pk, e_t):
                            h0 = 4 * pk + pg
                            for it, (ioff, icnt) in enumerate(TOK):
                                for hh in range(2):
                                    h = h0 + 2 * hh
                                    col = (2 * pk + hh) * 65
                                    for jt, (joff, jcnt) in enumerate(TOK):
                                        nc.tensor.matmul(
                                            av[it][:icnt, col : col + 65],
                                            e_t[jt][:jcnt, hh * N + ioff : hh * N + ioff + icnt],
                                            v_sb[bi][jt][:jcnt, h * 65 : h * 65 + 65],
                                            start=(pk == 0 and hh == 0 and jt == 0),
                                            stop=(pk == 2 and hh == 1 and jt == 1),
                                        )

                        prev = None
                        for pk in range(4):  # 3 pairs + AV lagged one pair
                            if pk < 3:
                                h0 = 4 * pk + pg
                                e_t = []
                                for jt, (joff, jcnt) in enumerate(TOK):
                                    ps = ps_big.tile([128, GW], F32, tag="big", name="pssc")
                                    for hh in range(2):
                                        h = h0 + 2 * hh
                                        mo = h // 2
                                        nc.tensor.matmul(
                                            ps[:jcnt, hh * N : (hh + 1) * N],
                                            kTb[mo][
                                                ro : ro + 64,
                                                bi * N + joff : bi * N + joff + jcnt,
                                            ],
                                            qTb[mo][ro : ro + 64, bi * N : bi * N + N],
                                            start=(hh == 0),
                                            stop=False,
                                        )
                                    nc.tensor.matmul(
                                        ps[:jcnt, :],
                                        identb[:jcnt, :jcnt],
                                        bt_t[bi, pg, pk, jt][:jcnt, :],
                                        start=False,
                                        stop=True,
                                    )
                                    e = epool.tile(
                                        [128, GW], BF16, tag=f"e{jt}", name=f"e{jt}"
                                    )
                                    nc.scalar.activation(e[:jcnt, :], ps[:jcnt, :], AF.Exp)
                                    e_t.append(e)
                            if pk >= 1:
                                av_block(*prev)
                            prev = (pk, e_t) if pk < 3 else None
                        # normalize 6 heads at once per token tile
                        for it, (ioff, icnt) in enumerate(TOK):
                            av3 = av[it][:icnt, :].rearrange("p (h c) -> p h c", c=65)
                            rr = spool.tile([128, 6], F32, tag="rr")
                            nc.vector.reciprocal(rr[:icnt, :].unsqueeze(2), av3[:, :, 64:65])
                            ov = out_sb[bi][it][:icnt, :].rearrange(
                                "p (k two d) -> p k two d", two=2, d=64
                            )[:, :, pg, :]
                            nc.vector.tensor_mul(
                                ov,
                                av3[:, :, 0:64],
                                rr[:icnt, :].unsqueeze(2).broadcast_to([icnt, 6, 64]),
                            )
                for bi in range(2):
                    for it, (toff, tcnt) in enumerate(TOK):
                        nc.gpsimd.dma_start(
                            outc[2 * g + bi, toff : toff + tcnt, :], out_sb[bi][it][:tcnt, :]
                        )
    nc.compile()
    return nc


_NC_CACHE = None


def _get_nc():
    global _NC_CACHE
    if _NC_CACHE is None:
        _NC_CACHE = build_nc()
    return _NC_CACHE


def _prep_inputs(x, Wq, Wk, Wv, W_var, b_var, W_alpha, b_alpha, diff):
    import ml_dtypes

    bf16 = ml_dtypes.bfloat16
    x = np.asarray(x, np.float32)
    wq = np.ascontiguousarray(np.asarray(Wq, np.float32).T).astype(bf16)
    wk = np.ascontiguousarray(np.asarray(Wk, np.float32).T * 0.125).astype(bf16)
    wv = np.ascontiguousarray(np.asarray(Wv, np.float32).T).astype(bf16)
    W_var = np.asarray(W_var, np.float32)
    W_alpha = np.asarray(W_alpha, np.float32)
    diff = np.asarray(diff)
    # block-diagonal [768, 36]: cols 3h+{0,1,2} = W_var[0], W_var[1], W_alpha
    wva = np.zeros((E, 36), np.float32)
    for h in range(H):
        sl = slice(h * DH, (h + 1) * DH)
        wva[sl, 3 * h + 0] = W_var[0]
        wva[sl, 3 * h + 1] = W_var[1]
        wva[sl, 3 * h + 2] = W_alpha[0]
    wva = wva.astype(bf16)
    # grid coordinates per token (derived from diff against patch 0 at (0,0))
    pxp = np.sqrt(diff[:, 0, 0].astype(np.float64)).astype(np.float32)  # (196,)
    pyp = np.sqrt(diff[:, 0, 1].astype(np.float64)).astype(np.float32)
    px = np.concatenate([[0.0], pxp]).astype(np.float32)  # (197,) token-indexed
    py = np.concatenate([[0.0], pyp]).astype(np.float32)
    # L6 [6, 197]: col j>=1 -> [1, px, px^2, py, py^2, 0]; col 0 (cls) -> e_5
    l6a = np.zeros((6, N), np.float32)
    l6a[0, 1:] = 1.0
    l6a[1, 1:] = px[1:]
    l6a[2, 1:] = px[1:] ** 2
    l6a[3, 1:] = py[1:]
    l6a[4, 1:] = py[1:] ** 2
    l6a[5, 0] = 1.0
    # 12 block lhsT variants: l6[h] has L6 at rows 6h..6h+5, zeros elsewhere
    l6 = np.zeros((H, 72, N), np.float32)
    for h in range(H):
        l6[h, 6 * h : 6 * h + 6] = l6a
    l6 = l6.astype(np.float16)
    p2 = np.stack([px, px**2, py, py**2], axis=1).astype(np.float32)  # (197, 4)
    bias3 = np.tile(
        np.concatenate([np.asarray(b_var, np.float32), np.asarray(b_alpha, np.float32)]),
        (128, H),
    ).astype(np.float32)
    shared = dict(wq=wq, wk=wk, wv=wv, wva=wva, l6=l6, p2=p2, bias3=bias3)
    in_maps = []
    for c in range(NCORES):
        m = dict(shared)
        m["xc"] = np.ascontiguousarray(x[c * BPC : (c + 1) * BPC])
        in_maps.append(m)
    return in_maps


def run(trace=False, **inputs):
    nc = _get_nc()
    in_maps = _prep_inputs(**inputs)
    res = run_bass_kernel_spmd(nc, in_maps, list(range(NCORES)), trace=trace)
    out = np.concatenate([res.results[c]["outc"] for c in range(NCORES)], axis=0)
    return out, res


def kernel(**inputs):
    out, _ = run(trace=False, **inputs)
    return out


# revision 31
# speedup vs baseline: 1.4166x; 1.2169x over previous
import sys

import numpy as np

for _p in ("/opt/trn_rl_repo",):
    if _p not in sys.path:
        sys.path.insert(0, _p)

import concourse.bass as bass
import concourse.mybir as mybir
from concourse import bacc
import concourse.tile as tile
from concourse import masks
from concourse.bass_utils import run_bass_kernel_spmd

B, N, E, H, DH = 64, 197, 768, 12, 64
NCORES = 8
BPC = B // NCORES  # batches per core
EPS = 1e-6
F32 = mybir.dt.float32
F16 = mybir.dt.float16
BF16 = mybir.dt.bfloat16

# token partition tiles (all 197 tokens incl cls)
TOK = ((0, 128), (128, 69))
GROUPS = BPC // 2  # 2 batches per group
GW = 2 * N  # 394
AF = mybir.ActivationFunctionType


def build_nc():
    nc = bacc.Bacc()
    xc = nc.declare_dram_parameter("xc", [BPC, N, E], F32, isOutput=False)
    wq = nc.declare_dram_parameter("wq", [E, E], BF16, isOutput=False)
    wk = nc.declare_dram_parameter("wk", [E, E], BF16, isOutput=False)
    wv = nc.declare_dram_parameter("wv", [E, E], BF16, isOutput=False)
    wva = nc.declare_dram_parameter("wva", [E, 36], BF16, isOutput=False)
    # l6[h] = L6 block at rows 6h..6h+5, zeros elsewhere (K=72 lhsT variants,
    # sidesteps the PE base-partition-must-be-0/32/64 rule)
    l6 = nc.declare_dram_parameter("l6", [H, 72, N], F16, isOutput=False)
    p2 = nc.declare_dram_parameter("p2", [N, 4], F32, isOutput=False)
    bias3 = nc.declare_dram_parameter("bias3", [128, 36], F32, isOutput=False)
    outc = nc.declare_dram_parameter("outc", [BPC, N, E], F32, isOutput=True)

    with tile.TileContext(nc) as tc:
        from contextlib import ExitStack

        with ExitStack() as ctx:
            ep = ctx.enter_context

            cpool = ep(tc.tile_pool(name="const", bufs=1))
            trawpool = ep(tc.tile_pool(name="traw", bufs=2))
            xTpool = ep(tc.tile_pool(name="xT", bufs=2))
            qkpool = ep(tc.tile_pool(name="qk", bufs=2))
            vpool = ep(tc.tile_pool(name="v", bufs=2))
            spool = ep(tc.tile_pool(name="small", bufs=2))
            rpool = ep(tc.tile_pool(name="r", bufs=4))
            btpool = ep(tc.tile_pool(name="bt", bufs=2))
            epool = ep(tc.tile_pool(name="e", bufs=3))
            opool = ep(tc.tile_pool(name="out", bufs=2))

            # PSUM banks: big 2 + arg 2 + av 2x2 = 8
            ps_big = ep(tc.tile_pool(name="ps_big", bufs=2, space="PSUM"))
            ps_arg = ep(tc.tile_pool(name="ps_arg", bufs=2, space="PSUM"))
            ps_av = ep(tc.tile_pool(name="ps_av", bufs=2, space="PSUM"))

            # ---- constants ----
            identf = cpool.tile([128, 128], F32, tag="identf")
            masks.make_identity(nc, identf[:, :])
            nc.vector.tensor_scalar_add(identf[:, :], identf[:, :], 0.0)
            identb = cpool.tile([128, 128], BF16, tag="identb")
            masks.make_identity(nc, identb[:, :])
            nc.vector.tensor_scalar_add(identb[:, :], identb[:, :], 0.0)
            identh = cpool.tile([128, 128], F16, tag="identh")
            masks.make_identity(nc, identh[:, :])
            nc.vector.tensor_scalar_add(identh[:, :], identh[:, :], 0.0)

            def emit_x_dma(g):
                traw = [[None, None], [None, None]]
                for bi in range(2):
                    for tb, (toff, tcnt) in enumerate(TOK):
                        t = trawpool.tile(
                            [128, E], F32, tag=f"tr{bi}{tb}", name=f"tr{bi}{tb}"
                        )
                        nc.gpsimd.dma_start(
                            t[:tcnt, :], xc[2 * g + bi, toff : toff + tcnt, :]
                        )
                        traw[bi][tb] = t
                return traw

            # x of group 0 first so PE transposes can start during weight DMA
            traw0 = emit_x_dma(0)

            # weights: one DMA per matrix into a [128, 6*768] tile (ke chunk at
            # cols ke*768), staged through DVE for matmul wait-slot hygiene
            w_big = {}
            for name, dram in (("q", wq), ("k", wk), ("v", wv)):
                raw = cpool.tile([128, 6 * E], BF16, tag=f"wr{name}", name=f"wr{name}")
                nc.gpsimd.dma_start(
                    raw[:, :].rearrange("p (ke f) -> p ke f", f=E),
                    dram.rearrange("(ke p) f -> p ke f", p=128),
                )
                t = cpool.tile([128, 6 * E], BF16, tag=f"w{name}", name=f"w{name}")
                nc.vector.tensor_scalar_add(t[:, :], raw[:, :], 0.0)
                w_big[name] = t
            wvar = cpool.tile([128, 6 * 36], BF16, tag="wvar")
            nc.gpsimd.dma_start(
                wvar[:, :].rearrange("p (ke f) -> p ke f", f=36),
                wva.rearrange("(ke p) f -> p ke f", p=128),
            )
            wva_t = cpool.tile([128, 6 * 36], BF16, tag="wvat")
            nc.vector.tensor_scalar_add(wva_t[:, :], wvar[:, :], 0.0)
            l6r = cpool.tile([72, H * N], F16, tag="l6r")
            nc.gpsimd.dma_start(
                l6r[:, :].rearrange("p (h n) -> p h n", n=N),
                l6.rearrange("h p n -> p h n"),
            )
            l6_t = cpool.tile([72, H * N], F16, tag="l6t")
            nc.vector.tensor_scalar_add(l6_t[:, :], l6r[:, :], 0.0)
            p2_t = []
            for tt, (toff, tcnt) in enumerate(TOK):
                t = cpool.tile([128, 4], F32, tag=f"p2{tt}")
                nc.gpsimd.dma_start(t[:tcnt, :], p2[toff : toff + tcnt, :])
                p2_t.append(t)
            bias_t = cpool.tile([128, 36], F32, tag="bias3")
            nc.gpsimd.dma_start(bias_t[:, :], bias3[:, :])

            def prep_blocks(g, traw, st):
                """Generator: x transpose + q/k projection PE blocks for group g.

                Yields after each PSUM-allocating block so the caller can
                interleave these dense chains into the previous group's
                attention stream (keeps PE activity high -> HAM stays warm).
                """
                xT = st["xT"]
                for eb in range(6):
                    tp = ps_big.tile([128, GW], F32, tag="big", name="tpx")
                    for idx, (bi, tb) in enumerate(
                        [(b, t) for b in range(2) for t in range(2)]
                    ):
                        toff, tcnt = TOK[tb]
                        nc.tensor.matmul(
                            tp[:128, bi * N + toff : bi * N + toff + tcnt],
                            traw[bi][tb][:tcnt, eb * 128 : (eb + 1) * 128],
                            identf[:tcnt, :tcnt],
                            is_transpose=True,
                            start=(idx == 0),
                            stop=(idx == 3),
                        )
                    t = xTpool.tile([128, GW], BF16, tag=f"xT{eb}", name=f"xT{eb}")
                    nc.vector.tensor_scalar_add(t[:, :], tp[:, :], 0.0)
                    xT.append(t)
                    yield
                for nm in ("q", "k"):
                    wb = w_big[nm]
                    for mo in range(6):
                        ps = ps_big.tile([128, GW], F32, tag="big", name="psqk")
                        for ke in range(6):
                            nc.tensor.matmul(
                                ps[:, :],
                                wb[:, ke * E + mo * 128 : ke * E + (mo + 1) * 128],
                                xT[ke][:, :],
                                start=(ke == 0),
                                stop=(ke == 5),
                            )
                        t = qkpool.tile(
                            [128, GW], BF16, tag=f"{nm}T{mo}", name=f"{nm}T{mo}"
                        )
                        nc.vector.tensor_scalar_add(t[:, :], ps[:, :], 0.0)
                        st[nm].append(t)
                        yield

            st0 = {"xT": [], "q": [], "k": []}
            for _ in prep_blocks(0, traw0, st0):
                pass
            states = {0: st0}

            # ---- main loop over 2-batch groups ----
            for g in range(GROUPS):
                st = states[g]
                xT, qTb, kTb = st["xT"], st["q"], st["k"]

                # --- gaussian params -> R_T[bi] [72, 197] f16 (rows 6h+k) ---
                # ACT work is phase-batched (all Exp, then all Ln) to avoid
                # activation-table reloads between Exp and Ln.
                BP = [(bi, pt) for bi in range(2) for pt in range(2)]
                spe_t, sp_t, lna_t = {}, {}, {}
                for bi, pt in BP:
                    poff, pcnt = TOK[pt]
                    p36 = ps_arg.tile([128, 36], F32, tag="arg", name="p36")
                    for ke in range(6):
                        nc.tensor.matmul(
                            p36[:pcnt, :],
                            qTb[ke][:, bi * N + poff : bi * N + poff + pcnt],
                            wva_t[:, ke * 36 : (ke + 1) * 36],
                            start=(ke == 0),
                            stop=(ke == 5),
                        )
                    # softplus(x) = ln(1 + exp(x))
                    spa = spool.tile([128, 36], F32, tag=f"spa{bi}{pt}")
                    nc.vector.tensor_add(spa[:pcnt, :], p36[:pcnt, :], bias_t[:pcnt, :])
                    spe = spool.tile([128, 36], F32, tag=f"spe{bi}{pt}")
                    nc.scalar.activation(spe[:pcnt, :], spa[:pcnt, :], AF.Exp)
                    spe_t[bi, pt] = spe
                for bi, pt in BP:
                    poff, pcnt = TOK[pt]
                    sp = spool.tile([128, 36], F32, tag=f"sp{bi}{pt}")
                    nc.scalar.activation(
                        sp[:pcnt, :], spe_t[bi, pt][:pcnt, :], AF.Ln, bias=1.0
                    )
                    sp_t[bi, pt] = sp
                    lna = spool.tile([128, 12], F32, tag=f"lna{bi}{pt}")
                    sp3 = sp[:pcnt, :].rearrange("p (h c) -> p h c", c=3)
                    nc.scalar.activation(lna[:pcnt, :].unsqueeze(2), sp3[:, :, 2:3], AF.Ln)
                    lna_t[bi, pt] = lna
                R_T = []
                for bi in range(2):
                    rtps = ps_arg.tile([72, N], F16, tag="arg", name="rtps")
                    for pt, (poff, pcnt) in enumerate(TOK):
                        sp3 = sp_t[bi, pt][:pcnt, :].rearrange("p (h c) -> p h c", c=3)
                        lna = lna_t[bi, pt]
                        # rv[p, 2h+c] = 1/(softplus + 2eps)
                        rv = spool.tile([128, 24], F32, tag="rv")
                        rv3 = rv[:pcnt, :].rearrange("p (h c) -> p h c", c=2)
                        nc.vector.tensor_scalar_add(rv3, sp3[:, :, 0:2], 2.0 * EPS)
                        nc.vector.reciprocal(rv[:pcnt, :], rv[:pcnt, :])
                        rvx = rv3[:, :, 0:1]
                        rvy = rv3[:, :, 1:2]
                        # R rows per head: [lna-0.5(rvx*px^2+rvy*py^2), rvx*px,
                        #                   -0.5rvx, rvy*py, -0.5rvy, -40]
                        px = p2_t[pt][:pcnt, 0:1]
                        px2 = p2_t[pt][:pcnt, 1:2]
                        py = p2_t[pt][:pcnt, 2:3]
                        py2 = p2_t[pt][:pcnt, 3:4]
                        rpre = rpool.tile([128, 72], F16, tag="rpre")
                        r6 = rpre[:pcnt, :].rearrange("p (h k) -> p h k", k=6)
                        nc.vector.tensor_scalar_mul(r6[:, :, 1:2], rvx, px)
                        nc.vector.tensor_scalar_mul(r6[:, :, 3:4], rvy, py)
                        nc.vector.tensor_scalar_mul(r6[:, :, 2:3], rvx, -0.5)
                        nc.vector.tensor_scalar_mul(r6[:, :, 4:5], rvy, -0.5)
                        ta = spool.tile([128, 12], F32, tag="ta")
                        tb2 = spool.tile([128, 12], F32, tag="tb2")
                        nc.vector.tensor_scalar_mul(ta[:pcnt, :].unsqueeze(2), rvx, px2)
                        nc.vector.tensor_scalar_mul(tb2[:pcnt, :].unsqueeze(2), rvy, py2)
                        tc2 = spool.tile([128, 12], F32, tag="tc2")
                        nc.vector.tensor_add(tc2[:pcnt, :], ta[:pcnt, :], tb2[:pcnt, :])
                        nc.vector.tensor_scalar_mul(tc2[:pcnt, :], tc2[:pcnt, :], -0.5)
                        nc.vector.tensor_add(
                            r6[:, :, 0:1],
                            tc2[:pcnt, :].unsqueeze(2),
                            lna[:pcnt, :].unsqueeze(2),
                        )
                        nc.vector.memset(r6[:, :, 5:6], -40.0)
                        if pt == 0:
                            # cls query col: zero linear terms, force R0 (and keep
                            # R5) at -40 so bias underflows to 0 for i=0 and (0,0)
                            r60 = rpre[0:1, :].rearrange("p (h k) -> p h k", k=6)
                            nc.vector.memset(r60[:, :, 0:5], 0.0)
                            nc.vector.memset(r60[:, :, 0:1], -40.0)
                        nc.tensor.matmul(
                            rtps[:72, poff : poff + pcnt],
                            rpre[:pcnt, :72],
                            identh[:pcnt, :pcnt],
                            is_transpose=True,
                            start=(pt == 0),
                            stop=(pt == 1),
                        )
                    t = rpool.tile([72, N], F16, tag="rT", name="rT")
                    nc.vector.tensor_scalar_add(t[:, :], rtps[:, :], 0.0)
                    R_T.append(t)

                # --- v natural layout with interleaved ones col: [tok, 12*65] ---
                v_sb = [[None, None], [None, None]]
                for bi in range(2):
                    for tb, (toff, tcnt) in enumerate(TOK):
                        t = vpool.tile(
                            [128, H * 65], BF16, tag=f"v{bi}{tb}", name=f"v{bi}{tb}"
                        )
                        tv = t[:tcnt, :].rearrange("p (h c) -> p h c", c=65)
                        for nb in range(2):
                            ps = ps_arg.tile([128, 384], F32, tag="arg", name="psv")
                            for ke in range(6):
                                nc.tensor.matmul(
                                    ps[:tcnt, :],
                                    xT[ke][:, bi * N + toff : bi * N + toff + tcnt],
                                    w_big["v"][
                                        :, ke * E + nb * 384 : ke * E + (nb + 1) * 384
                                    ],
                                    start=(ke == 0),
                                    stop=(ke == 5),
                                )
                            nc.vector.tensor_scalar_add(
                                tv[:, nb * 6 : (nb + 1) * 6, 0:64],
                                ps[:tcnt, :].rearrange("p (h c) -> p h c", c=64),
                                0.0,
                            )
                        nc.vector.memset(tv[:, :, 64:65], 1.0)
                        v_sb[bi][tb] = t

                # --- bias tiles: exp of the rank-6 arg matmul, all up front ---
                bt_t = {}
                for bi in range(2):
                    for pg in range(2):
                        for pk in range(3):
                            h0 = 4 * pk + pg
                            for jt, (joff, jcnt) in enumerate(TOK):
                                pa = ps_arg.tile([128, GW], F32, tag="arg", name="psarg")
                                for hh in range(2):
                                    h = h0 + 2 * hh
                                    nc.tensor.matmul(
                                        pa[:jcnt, hh * N : (hh + 1) * N],
                                        l6_t[:, h * N + joff : h * N + joff + jcnt],
                                        R_T[bi][:, :],
                                        start=(hh == 0),
                                        stop=(hh == 1),
                                    )
                                bt = btpool.tile(
                                    [128, GW], BF16, tag=f"bt{bi}{pg}{pk}{jt}", name="bt"
                                )
                                nc.scalar.activation(bt[:jcnt, :], pa[:jcnt, :], AF.Exp)
                                bt_t[bi, pg, pk, jt] = bt

                # next group's x DMA + prep generator, interleaved into attention
                if g + 1 < GROUPS:
                    traw1 = emit_x_dma(g + 1)
                    st1 = {"xT": [], "q": [], "k": []}
                    states[g + 1] = st1
                    prep_gen = prep_blocks(g + 1, traw1, st1)
                else:
                    prep_gen = None

                def interleave():
                    if prep_gen is not None:
                        next(prep_gen, None)

                # --- attention: same-parity head pairs (h, h+2) so both heads
                # share lhsT base partitions -> one PSUM bank per pair ---
                out_sb = [
                    [
                        opool.tile([128, E], F32, tag=f"o{bi}{it}", name=f"o{bi}{it}")
                        for it in range(2)
                    ]
                    for bi in range(2)
                ]
                for bi in range(2):
                    for pg in range(2):
                        ro = 64 * pg
                        av = [
                            ps_av.tile([128, 6 * 65], F32, tag=f"av{it}", name=f"av{it}")
                            for it in range(2)
                        ]

                        def av_block(pk, e_t):
                            h0 = 4 * pk + pg
                            for it, (ioff, icnt) in enumerate(TOK):
                                for hh in range(2):
                                    h = h0 + 2 * hh
                                    col = (2 * pk + hh) * 65
                                    for jt, (joff, jcnt) in enumerate(TOK):
                                        nc.tensor.matmul(
                                            av[it][:icnt, col : col + 65],
                                            e_t[jt][
                                                :jcnt, hh * N + ioff : hh * N + ioff + icnt
                                            ],
                                            v_sb[bi][jt][:jcnt, h * 65 : h * 65 + 65],
                                            start=(pk == 0 and hh == 0 and jt == 0),
                                            stop=(pk == 2 and hh == 1 and jt == 1),
                                        )

                        prev = None
                        for pk in range(4):  # 3 pairs + AV lagged one pair
                            if pk < 3:
                                h0 = 4 * pk + pg
                                e_t = []
                                for jt, (joff, jcnt) in enumerate(TOK):
                                    ps = ps_big.tile([128, GW], F32, tag="big", name="pssc")
                                    for hh in range(2):
                                        h = h0 + 2 * hh
                                        mo = h // 2
                                        nc.tensor.matmul(
                                            ps[:jcnt, hh * N : (hh + 1) * N],
                                            kTb[mo][
                                                ro : ro + 64,
                                                bi * N + joff : bi * N + joff + jcnt,
                                            ],
                                            qTb[mo][ro : ro + 64, bi * N : bi * N + N],
                                            start=(hh == 0),
                                            stop=False,
                                        )
                                    nc.tensor.matmul(
                                        ps[:jcnt, :],
                                        identb[:jcnt, :jcnt],
                                        bt_t[bi, pg, pk, jt][:jcnt, :],
                                        start=False,
                                        stop=True,
                                    )
                                    e = epool.tile(
                                        [128, GW], BF16, tag=f"e{jt}", name=f"e{jt}"
                                    )
                                    nc.scalar.activation(e[:jcnt, :], ps[:jcnt, :], AF.Exp)
                                    e_t.append(e)
                                    interleave()
                            if pk >= 1:
                                av_block(*prev)
                                interleave()
                            prev = (pk, e_t) if pk < 3 else None
                        # normalize 6 heads at once per token tile
                        for it, (ioff, icnt) in enumerate(TOK):
                            av3 = av[it][:icnt, :].rearrange("p (h c) -> p h c", c=65)
                            rr = spool.tile([128, 6], F32, tag="rr")
                            nc.vector.reciprocal(rr[:icnt, :].unsqueeze(2), av3[:, :, 64:65])
                            ov = out_sb[bi][it][:icnt, :].rearrange(
                                "p (k two d) -> p k two d", two=2, d=64
                            )[:, :, pg, :]
                            nc.vector.tensor_mul(
                                ov,
                                av3[:, :, 0:64],
                                rr[:icnt, :].unsqueeze(2).broadcast_to([icnt, 6, 64]),
                            )
                for bi in range(2):
                    for it, (toff, tcnt) in enumerate(TOK):
                        nc.gpsimd.dma_start(
                            outc[2 * g + bi, toff : toff + tcnt, :],
                            out_sb[bi][it][:tcnt, :],
                        )
                # flush any remaining prep blocks
                if prep_gen is not None:
                    for _ in prep_gen:
                        pass
    nc.compile()
    return nc


_NC_CACHE = None


def _get_nc():
    global _NC_CACHE
    if _NC_CACHE is None:
        _NC_CACHE = build_nc()
    return _NC_CACHE


def _prep_inputs(x, Wq, Wk, Wv, W_var, b_var, W_alpha, b_alpha, diff):
    import ml_dtypes

    bf16 = ml_dtypes.bfloat16
    x = np.asarray(x, np.float32)
    wq = np.ascontiguousarray(np.asarray(Wq, np.float32).T).astype(bf16)
    wk = np.ascontiguousarray(np.asarray(Wk, np.float32).T * 0.125).astype(bf16)
    wv = np.ascontiguousarray(np.asarray(Wv, np.float32).T).astype(bf16)
    W_var = np.asarray(W_var, np.float32)
    W_alpha = np.asarray(W_alpha, np.float32)
    diff = np.asarray(diff)
    # block-diagonal [768, 36]: cols 3h+{0,1,2} = W_var[0], W_var[1], W_alpha
    wva = np.zeros((E, 36), np.float32)
    for h in range(H):
        sl = slice(h * DH, (h + 1) * DH)
        wva[sl, 3 * h + 0] = W_var[0]
        wva[sl, 3 * h + 1] = W_var[1]
        wva[sl, 3 * h + 2] = W_alpha[0]
    wva = wva.astype(bf16)
    # grid coordinates per token (derived from diff against patch 0 at (0,0))
    pxp = np.sqrt(diff[:, 0, 0].astype(np.float64)).astype(np.float32)  # (196,)
    pyp = np.sqrt(diff[:, 0, 1].astype(np.float64)).astype(np.float32)
    px = np.concatenate([[0.0], pxp]).astype(np.float32)  # (197,) token-indexed
    py = np.concatenate([[0.0], pyp]).astype(np.float32)
    # L6 [6, 197]: col j>=1 -> [1, px, px^2, py, py^2, 0]; col 0 (cls) -> e_5
    l6a = np.zeros((6, N), np.float32)
    l6a[0, 1:] = 1.0
    l6a[1, 1:] = px[1:]
    l6a[2, 1:] = px[1:] ** 2
    l6a[3, 1:] = py[1:]
    l6a[4, 1:] = py[1:] ** 2
    l6a[5, 0] = 1.0
    # 12 block lhsT variants: l6[h] has L6 at rows 6h..6h+5, zeros elsewhere
    l6 = np.zeros((H, 72, N), np.float32)
    for h in range(H):
        l6[h, 6 * h : 6 * h + 6] = l6a
    l6 = l6.astype(np.float16)
    p2 = np.stack([px, px**2, py, py**2], axis=1).astype(np.float32)  # (197, 4)
    bias3 = np.tile(
        np.concatenate([np.asarray(b_var, np.float32), np.asarray(b_alpha, np.float32)]),
        (128, H),
    ).astype(np.float32)
    shared = dict(wq=wq, wk=wk, wv=wv, wva=wva, l6=l6, p2=p2, bias3=bias3)
    in_maps = []
    for c in range(NCORES):
        m = dict(shared)
        m["xc"] = np.ascontiguousarray(x[c * BPC : (c + 1) * BPC])
        in_maps.append(m)
    return in_maps


def run(trace=False, **inputs):
    nc = _get_nc()
    in_maps = _prep_inputs(**inputs)
    res = run_bass_kernel_spmd(nc, in_maps, list(range(NCORES)), trace=trace)
    out = np.concatenate([res.results[c]["outc"] for c in range(NCORES)], axis=0)
    return out, res


def kernel(**inputs):
    out, _ = run(trace=False, **inputs)
    return out
